# revision 17
# baseline (speedup 1.0000x reference)
"""
MLA attention (DeepSeek-style) on 8 TRN2 NeuronCores.

Sharding:
  phase 1 (LoRA-A projection + RMSNorm): sharded over sequence (256 rows/core).
    The host supplies hidden^T and wa pre-tiled in SBUF layout so every DMA
    moves long contiguous lines; the a-projection is computed directly in
    feature-major layout (latents^T = wa-as-lhsT @ hidden^T) -- no on-device
    transposes.  RMSNorm statistics (partition-dim sums) use squared tiles +
    a ones-matmul, a K=1 broadcast matmul and a fast approximate reciprocal.
    The kv+rope latent rows are computed, normalized and AllGathered first
    (early collective); the q latents follow.
  phase 2 (q/kv up-proj, attention, o_proj): sharded over heads (4 heads/core),
    w_o input-dim sharded; bf16 partial outputs summed on the host (the
    all-reduce).

All heavy matmuls run in bf16 with fp32 PSUM accumulation.  Everything is feature-major ("X^T" layout [feature, seq]) in phase 2:
  scores^T[sk, sq] from k^T/q^T; rope projections of head pairs share one
  128-row matmul; the softmax normalizer is a single ones-matmul over the
  DVE-accumulated probability sum per (head, sq-block); the causal mask is a
  multiplicative bf16 0/1 mask applied after exp; (A @ V)^T = matmul(lhsT=V,
  rhs=A^T); o_proj consumes (A@V)^T directly and writes bf16 partials.
DMA queues: sync streams wa/hidden, gpsimd stages collective inputs, loads
gathered kv latents + up-proj weights and writes the output; scalar carries
constants, gathered q latents and w_o, keeping its engine free for the RMS /
exp activation chain.
"""

import os
import sys
from contextlib import ExitStack

import numpy as np

for _p in ("/opt/trn_rl_repo", "/root/.axon_site/_ro/trn_rl_repo"):
    if os.path.isdir(_p) and _p not in sys.path:
        sys.path.insert(0, _p)

import ml_dtypes  # noqa: E402

import concourse.bacc as bacc  # noqa: E402
import concourse.bass as bass  # noqa: E402
import concourse.mybir as mybir  # noqa: E402
import concourse.tile as tile  # noqa: E402
from concourse.bass_utils import run_bass_kernel_spmd  # noqa: E402

# ---------------------------------------------------------------- constants
NCORES = 8
S = 2048
SL = S // NCORES  # 256 local rows in phase 1
HID = 4096
Q_LORA = 1536
KV_LORA = 512
ROPE = 64
C = Q_LORA + KV_LORA + ROPE  # 2112
CKV_R = KV_LORA + ROPE  # 576 kv+rope latent rows
NOPE = 128
V_DIM = 128
H = 32
HL = H // NCORES  # 4 local heads
NPAIR = HL // 2  # head pairs (rope matmuls fused per pair)
Q_HEAD = NOPE + ROPE  # 192
QB_COLS = 2 * Q_HEAD  # 384 packed cols per head pair
EPS = 1e-6

F32 = mybir.dt.float32
BF16 = mybir.dt.bfloat16

CQ_TILES = Q_LORA // 128  # 12
CKV_TILES = KV_LORA // 128  # 4
HT_TILES = HID // 128  # 32
S_TILES = S // 128  # 16
SQB = 512
NSQB = S // SQB  # 4
EB = 512
NEB = HID // EB  # 8
HTC = 8  # ht-tiles per streamed wa_q chunk
HTCK = 4  # ht-tiles per streamed wa_kv chunk (finer: faster first matmul)


# ---------------------------------------------------------------- program
def build_program() -> bass.Bass:
    nc = bacc.Bacc(
        "TRN2",
        target_bir_lowering=False,
        debug=False,
        num_devices=NCORES,
    )

    # hidT / wa arrive pre-tiled from the host in exact SBUF layout so each
    # DMA moves long contiguous lines at full HBM bandwidth
    hidT_d = nc.declare_dram_parameter("hidT", [128, HT_TILES * SL], BF16, isOutput=False)
    wakv_d = nc.declare_dram_parameter(
        "wa_kv", [128, HT_TILES * CKV_R], BF16, isOutput=False
    )
    waq_d = nc.declare_dram_parameter(
        "wa_q", [128, 2 * HT_TILES * 768], BF16, isOutput=False
    )
    wqb_d = nc.declare_dram_parameter(
        "wqb", [Q_LORA, NPAIR * QB_COLS], BF16, isOutput=False
    )
    wkvb_d = nc.declare_dram_parameter(
        "wkvb", [KV_LORA, HL * (NOPE + V_DIM)], BF16, isOutput=False
    )
    wo_d = nc.declare_dram_parameter("wo", [HL * V_DIM, HID], BF16, isOutput=False)
    mask_d = nc.declare_dram_parameter("mask", [4, 128, SQB], BF16, isOutput=False)
    ones_d = nc.declare_dram_parameter("ones", [128, 1], BF16, isOutput=False)
    onesr_d = nc.declare_dram_parameter("onesr", [1, 128], BF16, isOutput=False)
    out_d = nc.declare_dram_parameter("out", [S, HID], BF16, isOutput=True)

    # collective bounce buffers (internal DRAM)
    cc_in_kv = nc.dram_tensor("cc_in_kv", [CKV_R, SL], BF16)
    cc_out_kv = nc.dram_tensor(
        "cc_out_kv", [NCORES, CKV_R, SL], BF16, addr_space="Shared"
    )
    cc_in_q = nc.dram_tensor("cc_in_q", [Q_LORA, SL], BF16)
    cc_out_q = nc.dram_tensor(
        "cc_out_q", [NCORES, Q_LORA, SL], BF16, addr_space="Shared"
    )

    with tile.TileContext(nc, num_cores=NCORES) as tc, ExitStack() as stack:
        # ---------------- small persistent constants
        misc = stack.enter_context(tc.tile_pool(name="misc", bufs=1))
        ones_sb = misc.tile([128, 1], BF16, tag="ones", name="ones")
        onesr_sb = misc.tile([1, 128], BF16, tag="onesr", name="onesr")
        mask_sb = misc.tile([128, 4 * SQB], BF16, tag="mask", name="mask")
        eps_sb = misc.tile([128, 1], F32, tag="eps", name="eps")
        nc.gpsimd.memset(eps_sb[:], EPS)
        nc.scalar.dma_start(ones_sb[:], ones_d[:])
        nc.scalar.dma_start(onesr_sb[:], onesr_d[:])
        for d in range(4):
            nc.scalar.dma_start(mask_sb[:, d * SQB : (d + 1) * SQB], mask_d[d])

        # hidden^T: 4 chunks interleaved with the wa_kv chunks on the sync
        # queue so the first kv matmuls can start after ~2.5 MB of DMA
        hidT_pool = stack.enter_context(tc.tile_pool(name="hidT", bufs=1))
        hidT_sb = hidT_pool.tile([128, HT_TILES * SL], BF16, tag="hidT", name="hidT")

        # phase-2 weight tiles (DMAs issued on the gpsimd queue, after the kv
        # collective trigger)
        wkvb_pool = stack.enter_context(tc.tile_pool(name="wkvb", bufs=1))
        wkvb_sb = [
            wkvb_pool.tile(
                [128, HL * (NOPE + V_DIM)], BF16, tag=f"wkvb{kt}", name=f"wkvb{kt}"
            )
            for kt in range(CKV_TILES)
        ]
        wqb_pool = stack.enter_context(tc.tile_pool(name="wqb", bufs=1))
        wqb_sb = [
            wqb_pool.tile([128, NPAIR * QB_COLS], BF16, tag=f"wqb{kt}", name=f"wqb{kt}")
            for kt in range(CQ_TILES)
        ]

        # gathered kv latents (feature-major, full S)
        latkv = stack.enter_context(tc.tile_pool(name="latkv", bufs=1))
        latkv_sb = [
            latkv.tile([128, S], BF16, tag=f"latkv{i}", name=f"latkv{i}")
            for i in range(CKV_TILES)
        ]
        # k_pe duplicated into both partition halves (rope matmuls of odd
        # heads read rows 64:128 so lhsT/rhs base partitions match)
        kpe2 = latkv.tile([128, S], BF16, tag="kpe2", name="kpe2")

        # ---------------- phase 1: a-projection on local rows (feature-major)
        with ExitStack() as p1:
            raw_pool = p1.enter_context(tc.tile_pool(name="p1raw", bufs=1))
            # contiguous so each collective staging is a single DMA
            rawq = raw_pool.tile([128, CQ_TILES * SL], BF16, tag="rawq", name="rawq")
            rawkv = raw_pool.tile([128, CKV_TILES * SL], BF16, tag="rawkv", name="rawkv")
            rawrope = raw_pool.tile([64, SL], BF16, tag="rawrope", name="rawrope")
            sq_pool = p1.enter_context(tc.tile_pool(name="p1sq", bufs=3))
            f_pool = p1.enter_context(tc.tile_pool(name="p1f", bufs=1))
            psum_r = p1.enter_context(tc.tile_pool(name="p1r", bufs=1, space="PSUM"))
            psum_bc = p1.enter_context(tc.tile_pool(name="p1bc", bufs=1, space="PSUM"))

            def rms_sums(pr, raw, idx, start, stop, tag):
                sq = sq_pool.tile([128, SL], BF16, tag="sq", name=f"sq_{tag}{idx}")
                nc.vector.tensor_mul(
                    sq[:], raw[:, idx * SL : (idx + 1) * SL], raw[:, idx * SL : (idx + 1) * SL]
                )
                nc.tensor.matmul(pr[:], ones_sb[:], sq[:], start=start, stop=stop)

            def rms_finalize(pr, ncols, tag):
                """1/sqrt(mean(x^2)+eps) -> f32 [128, SL] broadcast tile."""
                sums = f_pool.tile([1, SL], BF16, tag=f"sums_{tag}", name=f"sums_{tag}")
                nc.scalar.copy(sums[:], pr[:])
                bc = psum_bc.tile([128, SL], F32, tag="bc", name=f"bc_{tag}")
                nc.tensor.matmul(bc[:], onesr_sb[:], sums[:], start=True, stop=True)
                ftmp = f_pool.tile([128, SL], F32, tag=f"ft_{tag}", name=f"ft_{tag}")
                nc.scalar.activation(
                    ftmp[:],
                    bc[:],
                    mybir.ActivationFunctionType.Sqrt,
                    scale=1.0 / ncols,
                    bias=eps_sb[:],
                )
                f = f_pool.tile([128, SL], F32, tag=f"f_{tag}", name=f"f_{tag}")
                nc.vector.reciprocal_approx_fast(f[:], ftmp[:])
                return f

            # ---- kv + rope latent rows first (early collective)
            with ExitStack() as p1kv:
                wakv_pool = p1kv.enter_context(tc.tile_pool(name="wakv", bufs=3))
                pkv_pool = p1kv.enter_context(
                    tc.tile_pool(name="pkv", bufs=1, space="PSUM")
                )
                pkv = [
                    pkv_pool.tile([128, SL], F32, tag=f"pkv{i}", name=f"pkv{i}")
                    for i in range(CKV_TILES)
                ]
                prope = pkv_pool.tile([64, SL], F32, tag="prope", name="prope")
                for htc in range(HT_TILES // HTCK):
                    nc.sync.dma_start(
                        hidT_sb[:, htc * HTCK * SL : (htc + 1) * HTCK * SL],
                        hidT_d[:, htc * HTCK * SL : (htc + 1) * HTCK * SL],
                    )
                    wt = wakv_pool.tile([128, HTCK * CKV_R], BF16, tag="wakv", name="wakv")
                    nc.sync.dma_start(
                        wt[:], wakv_d[:, htc * HTCK * CKV_R : (htc + 1) * HTCK * CKV_R]
                    )
                    for j in range(HTCK):
                        ht = htc * HTCK + j
                        rhs = hidT_sb[:, ht * SL : (ht + 1) * SL]
                        for i in range(CKV_TILES):
                            nc.tensor.matmul(
                                pkv[i][:],
                                wt[:, j * CKV_R + i * 128 : j * CKV_R + (i + 1) * 128],
                                rhs,
                                start=(ht == 0),
                                stop=(ht == HT_TILES - 1),
                            )
                        nc.tensor.matmul(
                            prope[:],
                            wt[:, j * CKV_R + KV_LORA : (j + 1) * CKV_R],
                            rhs,
                            start=(ht == 0),
                            stop=(ht == HT_TILES - 1),
                        )
                prkv = psum_r.tile([1, SL], F32, tag="pr", name="pr_kv")
                for i in range(CKV_TILES):
                    nc.vector.tensor_copy(rawkv[:, i * SL : (i + 1) * SL], pkv[i][:])
                    rms_sums(prkv, rawkv, i, i == 0, i == CKV_TILES - 1, "kv")
                nc.vector.tensor_copy(rawrope[:], prope[:])
                fkv = rms_finalize(prkv, KV_LORA, "kv")
                for i in range(CKV_TILES):
                    nc.vector.tensor_mul(
                        rawkv[:, i * SL : (i + 1) * SL],
                        rawkv[:, i * SL : (i + 1) * SL],
                        fkv[:],
                    )
                nc.scalar.dma_start(
                    cc_in_kv[:KV_LORA].rearrange("(c p) s -> p c s", p=128),
                    rawkv[:].rearrange("p (c s) -> p c s", c=CKV_TILES),
                )
                nc.scalar.dma_start(cc_in_kv[KV_LORA:, :], rawrope[:])
            nc.gpsimd.collective_compute(
                "AllGather",
                mybir.AluOpType.bypass,
                replica_groups=[list(range(NCORES))],
                ins=[cc_in_kv[:].opt()],
                outs=[cc_out_kv[:].opt()],
            )
            # gathered kv latents + up-proj weights on the gpsimd queue (its
            # engine has nothing latency-critical to issue until q staging)
            cc_kv_view = cc_out_kv[:].rearrange("j c s -> c j s")
            for i in range(CKV_TILES):
                nc.gpsimd.dma_start(
                    latkv_sb[i][:].rearrange("c (j s) -> c j s", j=NCORES),
                    cc_kv_view[i * 128 : (i + 1) * 128],
                )
            for half in range(2):
                nc.gpsimd.dma_start(
                    kpe2[half * 64 : (half + 1) * 64, :].rearrange(
                        "c (j s) -> c j s", j=NCORES
                    ),
                    cc_kv_view[KV_LORA:],
                )
            for kt in range(CKV_TILES):
                nc.gpsimd.dma_start(
                    wkvb_sb[kt][:], wkvb_d[kt * 128 : (kt + 1) * 128, :]
                )
            for kt in range(CQ_TILES):
                nc.gpsimd.dma_start(wqb_sb[kt][:], wqb_d[kt * 128 : (kt + 1) * 128, :])

            # ---- q latent rows (two column halves to bound PSUM usage; the
            # RMS square-sums of each half run while the other half computes)
            with ExitStack() as p1q:
                waq_pool = p1q.enter_context(tc.tile_pool(name="waq", bufs=4))
                pq_pool = p1q.enter_context(
                    tc.tile_pool(name="pq1", bufs=1, space="PSUM")
                )
                pq = [
                    pq_pool.tile([128, SL], F32, tag=f"pq{i}", name=f"pq{i}")
                    for i in range(6)
                ]
                prq = psum_r.tile([1, SL], F32, tag="pr", name="pr_q")
                for halfc in range(2):
                    for htc in range(HT_TILES // HTC):
                        wt = waq_pool.tile([128, HTC * 768], BF16, tag="waq", name="waq")
                        src0 = (halfc * HT_TILES + htc * HTC) * 768
                        nc.sync.dma_start(
                            wt[:], waq_d[:, src0 : src0 + HTC * 768]
                        )
                        for j in range(HTC):
                            ht = htc * HTC + j
                            rhs = hidT_sb[:, ht * SL : (ht + 1) * SL]
                            for i in range(6):
                                nc.tensor.matmul(
                                    pq[i][:],
                                    wt[:, j * 768 + i * 128 : j * 768 + (i + 1) * 128],
                                    rhs,
                                    start=(ht == 0),
                                    stop=(ht == HT_TILES - 1),
                                )
                    for i in range(6):
                        ct = halfc * 6 + i
                        nc.vector.tensor_copy(rawq[:, ct * SL : (ct + 1) * SL], pq[i][:])
                        rms_sums(prq, rawq, ct, ct == 0, ct == CQ_TILES - 1, "q")
                fq = rms_finalize(prq, Q_LORA, "q")
                for ct in range(CQ_TILES):
                    nc.vector.tensor_mul(
                        rawq[:, ct * SL : (ct + 1) * SL],
                        rawq[:, ct * SL : (ct + 1) * SL],
                        fq[:],
                    )
                nc.scalar.dma_start(
                    cc_in_q[:].rearrange("(c p) s -> p c s", p=128),
                    rawq[:].rearrange("p (c s) -> p c s", c=CQ_TILES),
                )
            nc.gpsimd.collective_compute(
                "AllGather",
                mybir.AluOpType.bypass,
                replica_groups=[list(range(NCORES))],
                ins=[cc_in_q[:].opt()],
                outs=[cc_out_q[:].opt()],
            )

        # ---------------- phase 2
        kvpool = stack.enter_context(tc.tile_pool(name="kvpool", bufs=1))
        knopeT = [
            kvpool.tile([128, S], BF16, tag=f"knopeT{h}", name=f"knopeT{h}")
            for h in range(HL)
        ]
        v_sb = [
            kvpool.tile([128, HL * V_DIM], BF16, tag=f"v{st}", name=f"v{st}")
            for st in range(S_TILES)
        ]

        with ExitStack() as p2kv:
            # ---- k_nope^T and V up-projections: depend only on the early kv
            # collective, so they overlap the q gather
            pkv_pool = p2kv.enter_context(tc.tile_pool(name="pkv2", bufs=4, space="PSUM"))
            for h in range(HL):
                for skb in range(NSQB):
                    pk = pkv_pool.tile([128, SQB], F32, tag="pkv", name="pk")
                    for kt in range(CKV_TILES):
                        nc.tensor.matmul(
                            pk[:],
                            wkvb_sb[kt][
                                :, h * (NOPE + V_DIM) : h * (NOPE + V_DIM) + NOPE
                            ],
                            latkv_sb[kt][:, skb * SQB : (skb + 1) * SQB],
                            start=(kt == 0),
                            stop=(kt == CKV_TILES - 1),
                        )
                    nc.vector.tensor_copy(
                        knopeT[h][:, skb * SQB : (skb + 1) * SQB], pk[:]
                    )
            for st in range(S_TILES):
                pv = pkv_pool.tile([128, HL * V_DIM], F32, tag="pkv", name="pv")
                for kt in range(CKV_TILES):
                    rhs = wkvb_sb[kt][:].rearrange("c (h d) -> c h d", h=HL)[:, :, NOPE:]
                    nc.tensor.matmul(
                        pv[:],
                        latkv_sb[kt][:, st * 128 : (st + 1) * 128],
                        rhs,
                        start=(kt == 0),
                        stop=(kt == CKV_TILES - 1),
                    )
                nc.vector.tensor_copy(v_sb[st][:], pv[:])

        qT = stack.enter_context(tc.tile_pool(name="qT", bufs=1))
        qTA = [qT.tile([128, S], BF16, tag=f"qTA{h}", name=f"qTA{h}") for h in range(HL)]
        # rope q of head pair p: rows 0:64 = head 2p, rows 64:128 = head 2p+1
        qTB = [
            qT.tile([128, S], BF16, tag=f"qTB{p}", name=f"qTB{p}")
            for p in range(NPAIR)
        ]
        outT_pool = stack.enter_context(tc.tile_pool(name="outT", bufs=1))
        outT = [
            outT_pool.tile([128, S], BF16, tag=f"outT{h}", name=f"outT{h}")
            for h in range(HL)
        ]

        # q^T up-projection (scoped: big q-latents released after)
        with ExitStack() as p2q:
            latq = p2q.enter_context(tc.tile_pool(name="latq", bufs=1))
            latq_sb = [
                latq.tile([128, S], BF16, tag=f"latq{ct}", name=f"latq{ct}")
                for ct in range(CQ_TILES)
            ]
            cc_q_view = cc_out_q[:].rearrange("j c s -> c j s")
            for ct in range(CQ_TILES):
                nc.scalar.dma_start(
                    latq_sb[ct][:].rearrange("c (j s) -> c j s", j=NCORES),
                    cc_q_view[ct * 128 : (ct + 1) * 128],
                )
            pq_pool = p2q.enter_context(tc.tile_pool(name="pq", bufs=8, space="PSUM"))
            for p in range(NPAIR):
                # packed col chunks: [h2p nope | h2p rope + h2p+1 rope | h2p+1 nope]
                chunks = [
                    (p * QB_COLS, qTA[2 * p]),
                    (p * QB_COLS + 128, qTB[p]),
                    (p * QB_COLS + 256, qTA[2 * p + 1]),
                ]
                for col0, dstt in chunks:
                    pqs = [
                        pq_pool.tile([128, SQB], F32, tag="pq", name=f"pq{sqb}")
                        for sqb in range(NSQB)
                    ]
                    for kt in range(CQ_TILES):
                        for sqb in range(NSQB):
                            nc.tensor.matmul(
                                pqs[sqb][:],
                                wqb_sb[kt][:, col0 : col0 + 128],
                                latq_sb[kt][:, sqb * SQB : (sqb + 1) * SQB],
                                start=(kt == 0),
                                stop=(kt == CQ_TILES - 1),
                            )
                    for sqb in range(NSQB):
                        nc.vector.tensor_copy(
                            dstt[:, sqb * SQB : (sqb + 1) * SQB], pqs[sqb][:]
                        )

        # w_o loaded after the q-latents scope closes; the scalar-queue DMAs
        # land during attention, well before o_proj
        wo_pool = stack.enter_context(tc.tile_pool(name="wo", bufs=1))
        wo_sb = [
            wo_pool.tile([128, HID], BF16, tag=f"wo{h}", name=f"wo{h}")
            for h in range(HL)
        ]
        for h in range(HL):
            nc.scalar.dma_start(wo_sb[h][:], wo_d[h * 128 : (h + 1) * 128, :])

        # ---------------- attention (causal, block-skipped)
        # Software-pipelined: the mask/accumulate/AV work for a score tile is
        # emitted two tiles behind its scores so the PE never waits on the
        # ACT-exp chain; the per-(head, sq-block) renormalization epilogue is
        # deferred by one pair.
        with ExitStack() as p2a:
            ps_pool = p2a.enter_context(tc.tile_pool(name="ps", bufs=4, space="PSUM"))
            psum_sum_pool = p2a.enter_context(
                tc.tile_pool(name="psums", bufs=1, space="PSUM")
            )
            psum_o_pool = p2a.enter_context(
                tc.tile_pool(name="psumo", bufs=2, space="PSUM")
            )
            psum_bc2 = p2a.enter_context(
                tc.tile_pool(name="psbc2", bufs=1, space="PSUM")
            )
            a_pool = p2a.enter_context(tc.tile_pool(name="apool", bufs=6))
            bc_pool = p2a.enter_context(tc.tile_pool(name="bcpool", bufs=2))
            asum_pool = p2a.enter_context(tc.tile_pool(name="asum", bufs=2))

            tile_q = []  # score tiles awaiting mask/accum/AV
            ep_q = []  # pairs awaiting the renormalization epilogue

            def drain_tile():
                a, h, bq, tk, nk, asum, po = tile_q.pop(0)
                d = tk - 4 * bq
                if d >= 0:
                    nc.vector.tensor_mul(
                        a[:], a[:], mask_sb[:, d * SQB : (d + 1) * SQB]
                    )
                if tk == 0:
                    nc.vector.tensor_copy(asum[:], a[:])
                else:
                    nc.vector.tensor_add(asum[:], asum[:], a[:])
                nc.tensor.matmul(
                    po[:],
                    v_sb[tk][:, h * V_DIM : (h + 1) * V_DIM],
                    a[:],
                    start=(tk == 0),
                    stop=(tk == nk - 1),
                )
                if tk == nk - 1:
                    psum = psum_sum_pool.tile([1, SQB], F32, tag="psums", name="rsum")
                    nc.tensor.matmul(
                        psum[:], ones_sb[:], asum[:], start=True, stop=True
                    )
                    rs = bc_pool.tile([1, SQB], BF16, tag="rs", name="rs")
                    nc.scalar.copy(rs[:], psum[:])
                    ep_q.append((h, bq, po, rs))

            def drain_epilogue():
                h, bq, po, rs = ep_q.pop(0)
                bc_ps = psum_bc2.tile([128, SQB], F32, tag="bc2", name="bc_ps")
                nc.tensor.matmul(bc_ps[:], onesr_sb[:], rs[:], start=True, stop=True)
                rbc = bc_pool.tile([128, SQB], F32, tag="rbc", name="rbc")
                nc.vector.reciprocal_approx_fast(rbc[:], bc_ps[:])
                nc.vector.tensor_mul(
                    outT[h][:, bq * SQB : (bq + 1) * SQB], po[:], rbc[:]
                )

            for h in range(HL):
                kpe_rows = kpe2[(h % 2) * 64 : (h % 2) * 64 + 64, :]
                qTB_rows = qTB[h // 2][(h % 2) * 64 : (h % 2) * 64 + 64, :]
                for bq in range(NSQB):
                    nk = 4 * (bq + 1)
                    asum = asum_pool.tile([128, SQB], BF16, tag="asum", name="asum")
                    po = psum_o_pool.tile([128, SQB], F32, tag="psumo", name="po")
                    for tk in range(nk):
                        ps = ps_pool.tile([128, SQB], F32, tag="ps", name="ps")
                        nc.tensor.matmul(
                            ps[:],
                            knopeT[h][:, tk * 128 : (tk + 1) * 128],
                            qTA[h][:, bq * SQB : (bq + 1) * SQB],
                            start=True,
                            stop=False,
                        )
                        nc.tensor.matmul(
                            ps[:],
                            kpe_rows[:, tk * 128 : (tk + 1) * 128],
                            qTB_rows[:, bq * SQB : (bq + 1) * SQB],
                            start=False,
                            stop=True,
                        )
                        a = a_pool.tile([128, SQB], BF16, tag="a", name="a")
                        nc.scalar.activation(
                            a[:], ps[:], mybir.ActivationFunctionType.Exp
                        )
                        tile_q.append((a, h, bq, tk, nk, asum, po))
                        while len(tile_q) > 3:
                            drain_tile()
                        while len(ep_q) > 1:
                            drain_epilogue()
            while tile_q:
                drain_tile()
            while ep_q:
                drain_epilogue()

        # ---------------- o_proj (bf16 partials: summed across cores on host)
        with ExitStack() as p2o:
            pe_pool = p2o.enter_context(tc.tile_pool(name="pe", bufs=4, space="PSUM"))
            stage_pool = p2o.enter_context(tc.tile_pool(name="stage", bufs=2))
            for st in range(S_TILES):
                stg = stage_pool.tile([128, HID], BF16, tag="stage", name="stg")
                for half in range(2):
                    for ebl in range(NEB // 2):
                        eb = half * (NEB // 2) + ebl
                        pe = pe_pool.tile([128, EB], F32, tag="pe", name="pe")
                        for h in range(HL):
                            nc.tensor.matmul(
                                pe[:],
                                outT[h][:, st * 128 : (st + 1) * 128],
                                wo_sb[h][:, eb * EB : (eb + 1) * EB],
                                start=(h == 0),
                                stop=(h == HL - 1),
                            )
                        nc.vector.tensor_copy(stg[:, eb * EB : (eb + 1) * EB], pe[:])
                    if st < S_TILES - 1:
                        nc.gpsimd.dma_start(
                            out_d[
                                st * 128 : (st + 1) * 128,
                                half * HID // 2 : (half + 1) * HID // 2,
                            ],
                            stg[:, half * HID // 2 : (half + 1) * HID // 2],
                        )
                    else:
                        for qq in range(2):
                            c0 = (half * 2 + qq) * (HID // 4)
                            nc.gpsimd.dma_start(
                                out_d[st * 128 : (st + 1) * 128, c0 : c0 + HID // 4],
                                stg[:, c0 : c0 + HID // 4],
                            )

    nc.compile()
    return nc


_PROGRAM_CACHE = {}


def _get_program() -> bass.Bass:
    if "nc" not in _PROGRAM_CACHE:
        _PROGRAM_CACHE["nc"] = build_program()
    return _PROGRAM_CACHE["nc"]


def _make_mask() -> np.ndarray:
    # multiplicative mask[d, p, f] for diagonal score tiles (applied after
    # exp): sk-tile tk = 4*bq + d; valid (sq >= sk) <=> f >= 128*d + p
    d = np.arange(4)[:, None, None]
    p = np.arange(128)[None, :, None]
    f = np.arange(SQB)[None, None, :]
    return np.where(f >= 128 * d + p, 1.0, 0.0).astype(ml_dtypes.bfloat16)


def prepare_inputs(
    hidden_states, w_qkv_a, q_a_gamma, w_q_b, kv_a_gamma, w_kv_b, w_o, b_o
):
    """Host-side prep: fold gammas + attention scale into B weights, pack the
    rope columns of head pairs, pre-tile hidT/wa, fp8-quantize w_o, slice per
    core."""
    bf = ml_dtypes.bfloat16
    hs = np.asarray(hidden_states, np.float32).reshape(S, HID)
    scale = float(Q_HEAD) ** -0.5
    wqb_eff = (
        np.asarray(w_q_b, np.float32)
        * np.asarray(q_a_gamma, np.float32)[:, None]
        * scale
    )
    wkvb_eff = (
        np.asarray(w_kv_b, np.float32) * np.asarray(kv_a_gamma, np.float32)[:, None]
    )
    wa_f = np.asarray(w_qkv_a, np.float32)
    # pre-tiled wa in SBUF layout: [p, ht, cols] flattened on the free axis
    wa_kv = np.ascontiguousarray(
        wa_f[:, Q_LORA:]
        .reshape(HT_TILES, 128, CKV_R)
        .transpose(1, 0, 2)
        .reshape(128, HT_TILES * CKV_R)
        .astype(bf)
    )
    wa_q = np.ascontiguousarray(
        wa_f[:, :Q_LORA]
        .reshape(HT_TILES, 128, 2, 768)
        .transpose(1, 2, 0, 3)  # [p, half, ht, 768]
        .reshape(128, 2 * HT_TILES * 768)
        .astype(bf)
    )
    mask = _make_mask()
    ones = np.ones((128, 1), bf)
    onesr = np.ones((1, 128), bf)

    wqb_r = wqb_eff.reshape(Q_LORA, H, Q_HEAD)
    wo_r = np.asarray(w_o, np.float32).reshape(H, V_DIM, HID)

    in_maps = []
    for c in range(NCORES):
        # hidden^T pre-tiled: [p, ht*SL] with element = hid[token, ht*128+p]
        hsT = np.ascontiguousarray(
            hs[c * SL : (c + 1) * SL]
            .T.reshape(HT_TILES, 128, SL)
            .transpose(1, 0, 2)
            .reshape(128, HT_TILES * SL)
            .astype(bf)
        )
        # packed wqb: per head pair [nope(2p) | rope(2p), rope(2p+1) | nope(2p+1)]
        wq_parts = []
        for p in range(NPAIR):
            h0 = c * HL + 2 * p
            h1 = h0 + 1
            wq_parts += [
                wqb_r[:, h0, :NOPE],
                wqb_r[:, h0, NOPE:],
                wqb_r[:, h1, NOPE:],
                wqb_r[:, h1, :NOPE],
            ]
        wqb_c = np.ascontiguousarray(np.concatenate(wq_parts, axis=1).astype(bf))
        wkvb_c = np.ascontiguousarray(
            wkvb_eff.reshape(KV_LORA, H, NOPE + V_DIM)[:, c * HL : (c + 1) * HL]
            .reshape(KV_LORA, HL * (NOPE + V_DIM))
            .astype(bf)
        )
        wo_c = np.ascontiguousarray(
            wo_r[c * HL : (c + 1) * HL].reshape(HL * V_DIM, HID).astype(bf)
        )
        in_maps.append(
            {
                "hidT": hsT,
                "wa_kv": wa_kv,
                "wa_q": wa_q,
                "wqb": wqb_c,
                "wkvb": wkvb_c,
                "wo": wo_c,
                "mask": mask,
                "ones": ones,
                "onesr": onesr,
            }
        )
    return in_maps


def kernel(**inputs) -> np.ndarray:
    in_maps = prepare_inputs(**inputs)
    nc = _get_program()
    res = run_bass_kernel_spmd(nc, in_maps, list(range(NCORES)))
    out = np.zeros((S, HID), np.float32)
    for r in res.results:
        out += np.asarray(r["out"], np.float32)
    out = out + np.asarray(inputs["b_o"], np.float32)[None, :]
    return out.reshape(1, S, HID)


# revision 18
# speedup vs baseline: 1.1467x; 1.1467x over previous
"""
MLA attention (DeepSeek-style) on 8 TRN2 NeuronCores.

Sharding:
  phase 1 (LoRA-A projection + RMSNorm): sharded over sequence (256 rows/core).
    The host supplies hidden^T and wa pre-tiled in SBUF layout so every DMA
    moves long contiguous lines; the a-projection is computed directly in
    feature-major layout (latents^T = wa-as-lhsT @ hidden^T) -- no on-device
    transposes.  RMSNorm statistics (partition-dim sums) use squared tiles +
    a ones-matmul, a K=1 broadcast matmul and a fast approximate reciprocal.
    The kv+rope latent rows are computed, normalized and AllGathered first
    (early collective); the q latents follow.
  phase 2 (q/kv up-proj, attention, o_proj): sharded over heads (4 heads/core),
    w_o input-dim sharded; bf16 partial outputs summed on the host (the
    all-reduce).

All heavy matmuls run in bf16 with fp32 PSUM accumulation.  Everything is feature-major ("X^T" layout [feature, seq]) in phase 2:
  scores^T[sk, sq] from k^T/q^T; rope projections of head pairs share one
  128-row matmul; the softmax normalizer is a single ones-matmul over the
  DVE-accumulated probability sum per (head, sq-block); the causal mask is a
  multiplicative bf16 0/1 mask applied after exp; (A @ V)^T = matmul(lhsT=V,
  rhs=A^T); o_proj consumes (A@V)^T directly and writes bf16 partials.
DMA queues: sync streams wa/hidden, gpsimd stages collective inputs, loads
gathered kv latents + up-proj weights and writes the output; scalar carries
constants, gathered q latents and w_o, keeping its engine free for the RMS /
exp activation chain.
"""

import os
import sys
from contextlib import ExitStack

import numpy as np

for _p in ("/opt/trn_rl_repo", "/root/.axon_site/_ro/trn_rl_repo"):
    if os.path.isdir(_p) and _p not in sys.path:
        sys.path.insert(0, _p)

import ml_dtypes  # noqa: E402

import concourse.bacc as bacc  # noqa: E402
import concourse.bass as bass  # noqa: E402
import concourse.mybir as mybir  # noqa: E402
import concourse.tile as tile  # noqa: E402
from concourse.bass_utils import run_bass_kernel_spmd  # noqa: E402

# ---------------------------------------------------------------- constants
NCORES = 8
S = 2048
SL = S // NCORES  # 256 local rows in phase 1
HID = 4096
Q_LORA = 1536
KV_LORA = 512
ROPE = 64
C = Q_LORA + KV_LORA + ROPE  # 2112
CKV_R = KV_LORA + ROPE  # 576 kv+rope latent rows
NOPE = 128
V_DIM = 128
H = 32
HL = H // NCORES  # 4 local heads
NPAIR = HL // 2  # head pairs (rope matmuls fused per pair)
Q_HEAD = NOPE + ROPE  # 192
QB_COLS = 2 * Q_HEAD  # 384 packed cols per head pair
EPS = 1e-6

F32 = mybir.dt.float32
BF16 = mybir.dt.bfloat16

CQ_TILES = Q_LORA // 128  # 12
CKV_TILES = KV_LORA // 128  # 4
HT_TILES = HID // 128  # 32
S_TILES = S // 128  # 16
SQB = 512
NSQB = S // SQB  # 4
EB = 512
NEB = HID // EB  # 8
HTC = 8  # ht-tiles per streamed wa_q chunk
HTCK = 4  # ht-tiles per streamed wa_kv chunk (finer: faster first matmul)


# ---------------------------------------------------------------- program
def build_program() -> bass.Bass:
    nc = bacc.Bacc(
        "TRN2",
        target_bir_lowering=False,
        debug=False,
        num_devices=NCORES,
    )

    # hidT / wa arrive pre-tiled from the host in exact SBUF layout so each
    # DMA moves long contiguous lines at full HBM bandwidth
    hidT_d = nc.declare_dram_parameter("hidT", [128, HT_TILES * SL], BF16, isOutput=False)
    wakv_d = nc.declare_dram_parameter(
        "wa_kv", [128, HT_TILES * CKV_R], BF16, isOutput=False
    )
    waq_d = nc.declare_dram_parameter(
        "wa_q", [128, 2 * HT_TILES * 768], BF16, isOutput=False
    )
    wqb_d = nc.declare_dram_parameter(
        "wqb", [Q_LORA, NPAIR * QB_COLS], BF16, isOutput=False
    )
    wkvb_d = nc.declare_dram_parameter(
        "wkvb", [KV_LORA, HL * (NOPE + V_DIM)], BF16, isOutput=False
    )
    wo_d = nc.declare_dram_parameter("wo", [HL * V_DIM, HID], BF16, isOutput=False)
    mask_d = nc.declare_dram_parameter("mask", [4, 128, SQB], BF16, isOutput=False)
    ones_d = nc.declare_dram_parameter("ones", [128, 1], BF16, isOutput=False)
    onesr_d = nc.declare_dram_parameter("onesr", [1, 128], BF16, isOutput=False)
    out_d = nc.declare_dram_parameter("out", [S, HID], BF16, isOutput=True)

    # collective bounce buffers (internal DRAM)
    cc_in_kv = nc.dram_tensor("cc_in_kv", [CKV_R, SL], BF16)
    cc_out_kv = nc.dram_tensor(
        "cc_out_kv", [NCORES, CKV_R, SL], BF16, addr_space="Shared"
    )
    cc_in_q = nc.dram_tensor("cc_in_q", [Q_LORA, SL], BF16)
    cc_out_q = nc.dram_tensor(
        "cc_out_q", [NCORES, Q_LORA, SL], BF16, addr_space="Shared"
    )

    with tile.TileContext(nc, num_cores=NCORES) as tc, ExitStack() as stack:
        # ---------------- small persistent constants
        misc = stack.enter_context(tc.tile_pool(name="misc", bufs=1))
        ones_sb = misc.tile([128, 1], BF16, tag="ones", name="ones")
        onesr_sb = misc.tile([1, 128], BF16, tag="onesr", name="onesr")
        mask_sb = misc.tile([128, 4 * SQB], BF16, tag="mask", name="mask")
        eps_sb = misc.tile([128, 1], F32, tag="eps", name="eps")
        nc.gpsimd.memset(eps_sb[:], EPS)
        nc.scalar.dma_start(ones_sb[:], ones_d[:])
        nc.scalar.dma_start(onesr_sb[:], onesr_d[:])
        for d in range(4):
            nc.scalar.dma_start(mask_sb[:, d * SQB : (d + 1) * SQB], mask_d[d])

        # hidden^T: 4 chunks interleaved with the wa_kv chunks on the sync
        # queue so the first kv matmuls can start after ~2.5 MB of DMA
        hidT_pool = stack.enter_context(tc.tile_pool(name="hidT", bufs=1))
        hidT_sb = hidT_pool.tile([128, HT_TILES * SL], BF16, tag="hidT", name="hidT")

        # phase-2 weight tiles (DMAs issued on the gpsimd queue, after the kv
        # collective trigger)
        wkvb_pool = stack.enter_context(tc.tile_pool(name="wkvb", bufs=1))
        wkvb_sb = [
            wkvb_pool.tile(
                [128, HL * (NOPE + V_DIM)], BF16, tag=f"wkvb{kt}", name=f"wkvb{kt}"
            )
            for kt in range(CKV_TILES)
        ]
        wqb_pool = stack.enter_context(tc.tile_pool(name="wqb", bufs=1))
        wqb_sb = [
            wqb_pool.tile([128, NPAIR * QB_COLS], BF16, tag=f"wqb{kt}", name=f"wqb{kt}")
            for kt in range(CQ_TILES)
        ]

        # gathered kv latents (feature-major, full S)
        latkv = stack.enter_context(tc.tile_pool(name="latkv", bufs=1))
        latkv_sb = [
            latkv.tile([128, S], BF16, tag=f"latkv{i}", name=f"latkv{i}")
            for i in range(CKV_TILES)
        ]
        # k_pe duplicated into both partition halves (rope matmuls of odd
        # heads read rows 64:128 so lhsT/rhs base partitions match)
        kpe2 = latkv.tile([128, S], BF16, tag="kpe2", name="kpe2")

        # ---------------- phase 1: a-projection on local rows (feature-major)
        with ExitStack() as p1:
            raw_pool = p1.enter_context(tc.tile_pool(name="p1raw", bufs=1))
            # contiguous so each collective staging is a single DMA
            rawq = raw_pool.tile([128, CQ_TILES * SL], BF16, tag="rawq", name="rawq")
            rawkv = raw_pool.tile([128, CKV_TILES * SL], BF16, tag="rawkv", name="rawkv")
            rawrope = raw_pool.tile([64, SL], BF16, tag="rawrope", name="rawrope")
            sq_pool = p1.enter_context(tc.tile_pool(name="p1sq", bufs=3))
            f_pool = p1.enter_context(tc.tile_pool(name="p1f", bufs=1))
            psum_r = p1.enter_context(tc.tile_pool(name="p1r", bufs=1, space="PSUM"))
            psum_bc = p1.enter_context(tc.tile_pool(name="p1bc", bufs=1, space="PSUM"))

            def rms_sums(pr, raw, idx, start, stop, tag):
                sq = sq_pool.tile([128, SL], BF16, tag="sq", name=f"sq_{tag}{idx}")
                nc.vector.tensor_mul(
                    sq[:], raw[:, idx * SL : (idx + 1) * SL], raw[:, idx * SL : (idx + 1) * SL]
                )
                nc.tensor.matmul(pr[:], ones_sb[:], sq[:], start=start, stop=stop)

            def rms_finalize(pr, ncols, tag):
                """1/sqrt(mean(x^2)+eps) -> f32 [128, SL] broadcast tile."""
                sums = f_pool.tile([1, SL], BF16, tag=f"sums_{tag}", name=f"sums_{tag}")
                nc.scalar.copy(sums[:], pr[:])
                bc = psum_bc.tile([128, SL], F32, tag="bc", name=f"bc_{tag}")
                nc.tensor.matmul(bc[:], onesr_sb[:], sums[:], start=True, stop=True)
                ftmp = f_pool.tile([128, SL], F32, tag=f"ft_{tag}", name=f"ft_{tag}")
                nc.scalar.activation(
                    ftmp[:],
                    bc[:],
                    mybir.ActivationFunctionType.Sqrt,
                    scale=1.0 / ncols,
                    bias=eps_sb[:],
                )
                f = f_pool.tile([128, SL], F32, tag=f"f_{tag}", name=f"f_{tag}")
                nc.vector.reciprocal_approx_fast(f[:], ftmp[:])
                return f

            # ---- kv + rope latent rows first (early collective)
            with ExitStack() as p1kv:
                wakv_pool = p1kv.enter_context(tc.tile_pool(name="wakv", bufs=3))
                pkv_pool = p1kv.enter_context(
                    tc.tile_pool(name="pkv", bufs=1, space="PSUM")
                )
                pkv = [
                    pkv_pool.tile([128, SL], F32, tag=f"pkv{i}", name=f"pkv{i}")
                    for i in range(CKV_TILES)
                ]
                prope = pkv_pool.tile([64, SL], F32, tag="prope", name="prope")
                for htc in range(HT_TILES // HTCK):
                    nc.sync.dma_start(
                        hidT_sb[:, htc * HTCK * SL : (htc + 1) * HTCK * SL],
                        hidT_d[:, htc * HTCK * SL : (htc + 1) * HTCK * SL],
                    )
                    wt = wakv_pool.tile([128, HTCK * CKV_R], BF16, tag="wakv", name="wakv")
                    nc.sync.dma_start(
                        wt[:], wakv_d[:, htc * HTCK * CKV_R : (htc + 1) * HTCK * CKV_R]
                    )
                    for j in range(HTCK):
                        ht = htc * HTCK + j
                        rhs = hidT_sb[:, ht * SL : (ht + 1) * SL]
                        for i in range(CKV_TILES):
                            nc.tensor.matmul(
                                pkv[i][:],
                                wt[:, j * CKV_R + i * 128 : j * CKV_R + (i + 1) * 128],
                                rhs,
                                start=(ht == 0),
                                stop=(ht == HT_TILES - 1),
                            )
                        nc.tensor.matmul(
                            prope[:],
                            wt[:, j * CKV_R + KV_LORA : (j + 1) * CKV_R],
                            rhs,
                            start=(ht == 0),
                            stop=(ht == HT_TILES - 1),
                        )
                prkv = psum_r.tile([1, SL], F32, tag="pr", name="pr_kv")
                for i in range(CKV_TILES):
                    nc.vector.tensor_copy(rawkv[:, i * SL : (i + 1) * SL], pkv[i][:])
                    rms_sums(prkv, rawkv, i, i == 0, i == CKV_TILES - 1, "kv")
                nc.vector.tensor_copy(rawrope[:], prope[:])
                fkv = rms_finalize(prkv, KV_LORA, "kv")
                for i in range(CKV_TILES):
                    nc.vector.tensor_mul(
                        rawkv[:, i * SL : (i + 1) * SL],
                        rawkv[:, i * SL : (i + 1) * SL],
                        fkv[:],
                    )
                nc.scalar.dma_start(
                    cc_in_kv[:KV_LORA].rearrange("(c p) s -> p c s", p=128),
                    rawkv[:].rearrange("p (c s) -> p c s", c=CKV_TILES),
                )
                nc.scalar.dma_start(cc_in_kv[KV_LORA:, :], rawrope[:])
            nc.gpsimd.collective_compute(
                "AllGather",
                mybir.AluOpType.bypass,
                replica_groups=[list(range(NCORES))],
                ins=[cc_in_kv[:].opt()],
                outs=[cc_out_kv[:].opt()],
            )
            # gathered kv latents + up-proj weights on the gpsimd queue (its
            # engine has nothing latency-critical to issue until q staging)
            cc_kv_view = cc_out_kv[:].rearrange("j c s -> c j s")
            for i in range(CKV_TILES):
                nc.gpsimd.dma_start(
                    latkv_sb[i][:].rearrange("c (j s) -> c j s", j=NCORES),
                    cc_kv_view[i * 128 : (i + 1) * 128],
                )
            for half in range(2):
                nc.gpsimd.dma_start(
                    kpe2[half * 64 : (half + 1) * 64, :].rearrange(
                        "c (j s) -> c j s", j=NCORES
                    ),
                    cc_kv_view[KV_LORA:],
                )
            for kt in range(CKV_TILES):
                nc.gpsimd.dma_start(
                    wkvb_sb[kt][:], wkvb_d[kt * 128 : (kt + 1) * 128, :]
                )
            for kt in range(CQ_TILES):
                nc.gpsimd.dma_start(wqb_sb[kt][:], wqb_d[kt * 128 : (kt + 1) * 128, :])

            # ---- q latent rows (two column halves to bound PSUM usage; the
            # RMS square-sums of each half run while the other half computes)
            with ExitStack() as p1q:
                waq_pool = p1q.enter_context(tc.tile_pool(name="waq", bufs=4))
                pq_pool = p1q.enter_context(
                    tc.tile_pool(name="pq1", bufs=1, space="PSUM")
                )
                pq = [
                    pq_pool.tile([128, SL], F32, tag=f"pq{i}", name=f"pq{i}")
                    for i in range(6)
                ]
                prq = psum_r.tile([1, SL], F32, tag="pr", name="pr_q")
                for halfc in range(2):
                    for htc in range(HT_TILES // HTC):
                        wt = waq_pool.tile([128, HTC * 768], BF16, tag="waq", name="waq")
                        src0 = (halfc * HT_TILES + htc * HTC) * 768
                        nc.sync.dma_start(
                            wt[:], waq_d[:, src0 : src0 + HTC * 768]
                        )
                        for j in range(HTC):
                            ht = htc * HTC + j
                            rhs = hidT_sb[:, ht * SL : (ht + 1) * SL]
                            for i in range(6):
                                nc.tensor.matmul(
                                    pq[i][:],
                                    wt[:, j * 768 + i * 128 : j * 768 + (i + 1) * 128],
                                    rhs,
                                    start=(ht == 0),
                                    stop=(ht == HT_TILES - 1),
                                )
                    for i in range(6):
                        ct = halfc * 6 + i
                        nc.vector.tensor_copy(rawq[:, ct * SL : (ct + 1) * SL], pq[i][:])
                        rms_sums(prq, rawq, ct, ct == 0, ct == CQ_TILES - 1, "q")
                fq = rms_finalize(prq, Q_LORA, "q")
                for ct in range(CQ_TILES):
                    nc.vector.tensor_mul(
                        rawq[:, ct * SL : (ct + 1) * SL],
                        rawq[:, ct * SL : (ct + 1) * SL],
                        fq[:],
                    )
                nc.scalar.dma_start(
                    cc_in_q[:].rearrange("(c p) s -> p c s", p=128),
                    rawq[:].rearrange("p (c s) -> p c s", c=CQ_TILES),
                )
            nc.gpsimd.collective_compute(
                "AllGather",
                mybir.AluOpType.bypass,
                replica_groups=[list(range(NCORES))],
                ins=[cc_in_q[:].opt()],
                outs=[cc_out_q[:].opt()],
            )

        # ---------------- phase 2
        kvpool = stack.enter_context(tc.tile_pool(name="kvpool", bufs=1))
        knopeT = [
            kvpool.tile([128, S], BF16, tag=f"knopeT{h}", name=f"knopeT{h}")
            for h in range(HL)
        ]
        v_sb = [
            kvpool.tile([128, HL * V_DIM], BF16, tag=f"v{st}", name=f"v{st}")
            for st in range(S_TILES)
        ]

        with ExitStack() as p2kv:
            # ---- k_nope^T and V up-projections: depend only on the early kv
            # collective, so they overlap the q gather
            pkv_pool = p2kv.enter_context(tc.tile_pool(name="pkv2", bufs=4, space="PSUM"))
            for h in range(HL):
                for skb in range(NSQB):
                    pk = pkv_pool.tile([128, SQB], F32, tag="pkv", name="pk")
                    for kt in range(CKV_TILES):
                        nc.tensor.matmul(
                            pk[:],
                            wkvb_sb[kt][
                                :, h * (NOPE + V_DIM) : h * (NOPE + V_DIM) + NOPE
                            ],
                            latkv_sb[kt][:, skb * SQB : (skb + 1) * SQB],
                            start=(kt == 0),
                            stop=(kt == CKV_TILES - 1),
                        )
                    nc.vector.tensor_copy(
                        knopeT[h][:, skb * SQB : (skb + 1) * SQB], pk[:]
                    )
            for st in range(S_TILES):
                pv = pkv_pool.tile([128, HL * V_DIM], F32, tag="pkv", name="pv")
                for kt in range(CKV_TILES):
                    rhs = wkvb_sb[kt][:].rearrange("c (h d) -> c h d", h=HL)[:, :, NOPE:]
                    nc.tensor.matmul(
                        pv[:],
                        latkv_sb[kt][:, st * 128 : (st + 1) * 128],
                        rhs,
                        start=(kt == 0),
                        stop=(kt == CKV_TILES - 1),
                    )
                nc.vector.tensor_copy(v_sb[st][:], pv[:])

        qT = stack.enter_context(tc.tile_pool(name="qT", bufs=1))
        qTA = [qT.tile([128, S], BF16, tag=f"qTA{h}", name=f"qTA{h}") for h in range(HL)]
        # rope q of head pair p: rows 0:64 = head 2p, rows 64:128 = head 2p+1
        qTB = [
            qT.tile([128, S], BF16, tag=f"qTB{p}", name=f"qTB{p}")
            for p in range(NPAIR)
        ]
        outT_pool = stack.enter_context(tc.tile_pool(name="outT", bufs=1))
        outT = [
            outT_pool.tile([128, S], BF16, tag=f"outT{h}", name=f"outT{h}")
            for h in range(HL)
        ]

        # q^T up-projection (scoped: big q-latents released after)
        with ExitStack() as p2q:
            latq = p2q.enter_context(tc.tile_pool(name="latq", bufs=1))
            latq_sb = [
                latq.tile([128, S], BF16, tag=f"latq{ct}", name=f"latq{ct}")
                for ct in range(CQ_TILES)
            ]
            cc_q_view = cc_out_q[:].rearrange("j c s -> c j s")
            for ct in range(CQ_TILES):
                nc.scalar.dma_start(
                    latq_sb[ct][:].rearrange("c (j s) -> c j s", j=NCORES),
                    cc_q_view[ct * 128 : (ct + 1) * 128],
                )
            pq_pool = p2q.enter_context(tc.tile_pool(name="pq", bufs=8, space="PSUM"))
            for p in range(NPAIR):
                # packed col chunks: [h2p nope | h2p rope + h2p+1 rope | h2p+1 nope]
                chunks = [
                    (p * QB_COLS, qTA[2 * p]),
                    (p * QB_COLS + 128, qTB[p]),
                    (p * QB_COLS + 256, qTA[2 * p + 1]),
                ]
                for col0, dstt in chunks:
                    pqs = [
                        pq_pool.tile([128, SQB], F32, tag="pq", name=f"pq{sqb}")
                        for sqb in range(NSQB)
                    ]
                    for kt in range(CQ_TILES):
                        for sqb in range(NSQB):
                            nc.tensor.matmul(
                                pqs[sqb][:],
                                wqb_sb[kt][:, col0 : col0 + 128],
                                latq_sb[kt][:, sqb * SQB : (sqb + 1) * SQB],
                                start=(kt == 0),
                                stop=(kt == CQ_TILES - 1),
                            )
                    for sqb in range(NSQB):
                        nc.vector.tensor_copy(
                            dstt[:, sqb * SQB : (sqb + 1) * SQB], pqs[sqb][:]
                        )

        # w_o loaded after the q-latents scope closes; the scalar-queue DMAs
        # land during attention, well before o_proj
        wo_pool = stack.enter_context(tc.tile_pool(name="wo", bufs=1))
        wo_sb = [
            wo_pool.tile([128, HID], BF16, tag=f"wo{h}", name=f"wo{h}")
            for h in range(HL)
        ]
        for h in range(HL):
            nc.scalar.dma_start(wo_sb[h][:], wo_d[h * 128 : (h + 1) * 128, :])

        # ---------------- attention (causal, block-skipped)
        # Software-pipelined: the mask/accumulate/AV work for a score tile is
        # emitted two tiles behind its scores so the PE never waits on the
        # ACT-exp chain; the per-(head, sq-block) renormalization epilogue is
        # deferred by one pair.
        with ExitStack() as p2a:
            ps_pool = p2a.enter_context(tc.tile_pool(name="ps", bufs=3, space="PSUM"))
            psum_sum_pool = p2a.enter_context(
                tc.tile_pool(name="psums", bufs=2, space="PSUM")
            )
            psum_o_pool = p2a.enter_context(
                tc.tile_pool(name="psumo", bufs=2, space="PSUM")
            )
            psum_bc2 = p2a.enter_context(
                tc.tile_pool(name="psbc2", bufs=1, space="PSUM")
            )
            a_pool = p2a.enter_context(tc.tile_pool(name="apool", bufs=6))
            bc_pool = p2a.enter_context(tc.tile_pool(name="bcpool", bufs=2))
            asum_pool = p2a.enter_context(tc.tile_pool(name="asum", bufs=2))

            tile_q = []  # score tiles awaiting mask/accum/AV
            ep_q = []  # pairs awaiting the renormalization epilogue

            def drain_tile():
                a, h, bq, tk, nk, asum, po = tile_q.pop(0)
                d = tk - 4 * bq
                if d >= 0:
                    nc.vector.tensor_mul(
                        a[:], a[:], mask_sb[:, d * SQB : (d + 1) * SQB]
                    )
                if tk == 0:
                    nc.vector.tensor_copy(asum[:], a[:])
                else:
                    nc.vector.tensor_add(asum[:], asum[:], a[:])
                nc.tensor.matmul(
                    po[:],
                    v_sb[tk][:, h * V_DIM : (h + 1) * V_DIM],
                    a[:],
                    start=(tk == 0),
                    stop=(tk == nk - 1),
                )
                if tk == nk - 1:
                    psum = psum_sum_pool.tile([1, SQB], F32, tag="psums", name="rsum")
                    nc.tensor.matmul(
                        psum[:], ones_sb[:], asum[:], start=True, stop=True
                    )
                    rs = bc_pool.tile([1, SQB], BF16, tag="rs", name="rs")
                    nc.scalar.copy(rs[:], psum[:])
                    ep_q.append((h, bq, po, rs))

            def drain_epilogue():
                h, bq, po, rs = ep_q.pop(0)
                bc_ps = psum_bc2.tile([128, SQB], F32, tag="bc2", name="bc_ps")
                nc.tensor.matmul(bc_ps[:], onesr_sb[:], rs[:], start=True, stop=True)
                rbc = bc_pool.tile([128, SQB], F32, tag="rbc", name="rbc")
                nc.vector.reciprocal_approx_fast(rbc[:], bc_ps[:])
                nc.vector.tensor_mul(
                    outT[h][:, bq * SQB : (bq + 1) * SQB], po[:], rbc[:]
                )

            for h in range(HL):
                kpe_rows = kpe2[(h % 2) * 64 : (h % 2) * 64 + 64, :]
                qTB_rows = qTB[h // 2][(h % 2) * 64 : (h % 2) * 64 + 64, :]
                for bq in range(NSQB):
                    nk = 4 * (bq + 1)
                    asum = asum_pool.tile([128, SQB], BF16, tag="asum", name="asum")
                    po = psum_o_pool.tile([128, SQB], F32, tag="psumo", name="po")
                    for tk in range(nk):
                        ps = ps_pool.tile([128, SQB], F32, tag="ps", name="ps")
                        nc.tensor.matmul(
                            ps[:],
                            knopeT[h][:, tk * 128 : (tk + 1) * 128],
                            qTA[h][:, bq * SQB : (bq + 1) * SQB],
                            start=True,
                            stop=False,
                        )
                        nc.tensor.matmul(
                            ps[:],
                            kpe_rows[:, tk * 128 : (tk + 1) * 128],
                            qTB_rows[:, bq * SQB : (bq + 1) * SQB],
                            start=False,
                            stop=True,
                        )
                        a = a_pool.tile([128, SQB], BF16, tag="a", name="a")
                        nc.scalar.activation(
                            a[:], ps[:], mybir.ActivationFunctionType.Exp
                        )
                        tile_q.append((a, h, bq, tk, nk, asum, po))
                        while len(tile_q) > 2:
                            drain_tile()
                        while len(ep_q) > 1:
                            drain_epilogue()
            while tile_q:
                drain_tile()
            while ep_q:
                drain_epilogue()

        # ---------------- o_proj (bf16 partials: summed across cores on host)
        with ExitStack() as p2o:
            pe_pool = p2o.enter_context(tc.tile_pool(name="pe", bufs=4, space="PSUM"))
            stage_pool = p2o.enter_context(tc.tile_pool(name="stage", bufs=2))
            for st in range(S_TILES):
                stg = stage_pool.tile([128, HID], BF16, tag="stage", name="stg")
                for half in range(2):
                    for ebl in range(NEB // 2):
                        eb = half * (NEB // 2) + ebl
                        pe = pe_pool.tile([128, EB], F32, tag="pe", name="pe")
                        for h in range(HL):
                            nc.tensor.matmul(
                                pe[:],
                                outT[h][:, st * 128 : (st + 1) * 128],
                                wo_sb[h][:, eb * EB : (eb + 1) * EB],
                                start=(h == 0),
                                stop=(h == HL - 1),
                            )
                        nc.vector.tensor_copy(stg[:, eb * EB : (eb + 1) * EB], pe[:])
                    if st < S_TILES - 1:
                        nc.gpsimd.dma_start(
                            out_d[
                                st * 128 : (st + 1) * 128,
                                half * HID // 2 : (half + 1) * HID // 2,
                            ],
                            stg[:, half * HID // 2 : (half + 1) * HID // 2],
                        )
                    else:
                        for qq in range(2):
                            c0 = (half * 2 + qq) * (HID // 4)
                            nc.gpsimd.dma_start(
                                out_d[st * 128 : (st + 1) * 128, c0 : c0 + HID // 4],
                                stg[:, c0 : c0 + HID // 4],
                            )

    nc.compile()
    return nc


_PROGRAM_CACHE = {}


def _get_program() -> bass.Bass:
    if "nc" not in _PROGRAM_CACHE:
        _PROGRAM_CACHE["nc"] = build_program()
    return _PROGRAM_CACHE["nc"]


def _make_mask() -> np.ndarray:
    # multiplicative mask[d, p, f] for diagonal score tiles (applied after
    # exp): sk-tile tk = 4*bq + d; valid (sq >= sk) <=> f >= 128*d + p
    d = np.arange(4)[:, None, None]
    p = np.arange(128)[None, :, None]
    f = np.arange(SQB)[None, None, :]
    return np.where(f >= 128 * d + p, 1.0, 0.0).astype(ml_dtypes.bfloat16)


def prepare_inputs(
    hidden_states, w_qkv_a, q_a_gamma, w_q_b, kv_a_gamma, w_kv_b, w_o, b_o
):
    """Host-side prep: fold gammas + attention scale into B weights, pack the
    rope columns of head pairs, pre-tile hidT/wa, fp8-quantize w_o, slice per
    core."""
    bf = ml_dtypes.bfloat16
    hs = np.asarray(hidden_states, np.float32).reshape(S, HID)
    scale = float(Q_HEAD) ** -0.5
    wqb_eff = (
        np.asarray(w_q_b, np.float32)
        * np.asarray(q_a_gamma, np.float32)[:, None]
        * scale
    )
    wkvb_eff = (
        np.asarray(w_kv_b, np.float32) * np.asarray(kv_a_gamma, np.float32)[:, None]
    )
    wa_f = np.asarray(w_qkv_a, np.float32)
    # pre-tiled wa in SBUF layout: [p, ht, cols] flattened on the free axis
    wa_kv = np.ascontiguousarray(
        wa_f[:, Q_LORA:]
        .reshape(HT_TILES, 128, CKV_R)
        .transpose(1, 0, 2)
        .reshape(128, HT_TILES * CKV_R)
        .astype(bf)
    )
    wa_q = np.ascontiguousarray(
        wa_f[:, :Q_LORA]
        .reshape(HT_TILES, 128, 2, 768)
        .transpose(1, 2, 0, 3)  # [p, half, ht, 768]
        .reshape(128, 2 * HT_TILES * 768)
        .astype(bf)
    )
    mask = _make_mask()
    ones = np.ones((128, 1), bf)
    onesr = np.ones((1, 128), bf)

    wqb_r = wqb_eff.reshape(Q_LORA, H, Q_HEAD)
    wo_r = np.asarray(w_o, np.float32).reshape(H, V_DIM, HID)

    in_maps = []
    for c in range(NCORES):
        # hidden^T pre-tiled: [p, ht*SL] with element = hid[token, ht*128+p]
        hsT = np.ascontiguousarray(
            hs[c * SL : (c + 1) * SL]
            .T.reshape(HT_TILES, 128, SL)
            .transpose(1, 0, 2)
            .reshape(128, HT_TILES * SL)
            .astype(bf)
        )
        # packed wqb: per head pair [nope(2p) | rope(2p), rope(2p+1) | nope(2p+1)]
        wq_parts = []
        for p in range(NPAIR):
            h0 = c * HL + 2 * p
            h1 = h0 + 1
            wq_parts += [
                wqb_r[:, h0, :NOPE],
                wqb_r[:, h0, NOPE:],
                wqb_r[:, h1, NOPE:],
                wqb_r[:, h1, :NOPE],
            ]
        wqb_c = np.ascontiguousarray(np.concatenate(wq_parts, axis=1).astype(bf))
        wkvb_c = np.ascontiguousarray(
            wkvb_eff.reshape(KV_LORA, H, NOPE + V_DIM)[:, c * HL : (c + 1) * HL]
            .reshape(KV_LORA, HL * (NOPE + V_DIM))
            .astype(bf)
        )
        wo_c = np.ascontiguousarray(
            wo_r[c * HL : (c + 1) * HL].reshape(HL * V_DIM, HID).astype(bf)
        )
        in_maps.append(
            {
                "hidT": hsT,
                "wa_kv": wa_kv,
                "wa_q": wa_q,
                "wqb": wqb_c,
                "wkvb": wkvb_c,
                "wo": wo_c,
                "mask": mask,
                "ones": ones,
                "onesr": onesr,
            }
        )
    return in_maps


def kernel(**inputs) -> np.ndarray:
    in_maps = prepare_inputs(**inputs)
    nc = _get_program()
    res = run_bass_kernel_spmd(nc, in_maps, list(range(NCORES)))
    out = np.zeros((S, HID), np.float32)
    for r in res.results:
        out += np.asarray(r["out"], np.float32)
    out = out + np.asarray(inputs["b_o"], np.float32)[None, :]
    return out.reshape(1, S, HID)
